# revision 13
# baseline (speedup 1.0000x reference)
"""Trainium2 Bass kernel for CuGraphRelGraphConv (basis-decomposed relational
graph conv) on 8 NeuronCores.

Math (reference):
    msg_e   = coeff[etype_e] (x) feat[src_e]            # [E, 2, 64]
    agg     = segment_sum(msg, dst)                     # [N, 2, 64]
    h       = agg.reshape(N,128) @ W.reshape(128,64) + bias + feat @ loop_w

Device mapping per core (dst-sharded, 12544 nodes/core, 196 windows x 64):
  - gather: per-edge rows of a bf16 table [N, 128] ([feat | 0] padding to
    256B) via gpsimd.dma_gather, int16 indices bucketed into 4 banks of
    2^15 rows (src = bank*32768 + idx16).
  - scaled selection: S01[e, (b, slot)] = coeff[et_e, b] * (dstl_e == slot)
    built on DVE from host-precomputed per-edge (dstl, c0, c1).
  - scatter matmul per 128-edge tile: PSUM agg[d, (b,slot)] += G_t^T @ S01_t
  - per window: h[slot, o] = sum_b agg_b^T-free matmuls with W[b] + fused
    self-loop + bias via an extra ones-row in featT.
Edges, windows and bank buckets are bucketed/padded on host; the static
schedule (slot counts) is the max over the 8 cores so one SPMD program
serves all cores.
"""
import sys

sys.path.insert(0, "/opt/trn_rl_repo")

import numpy as np
import ml_dtypes

import concourse.bass as bass
import concourse.bacc as bacc
import concourse.mybir as mybir
from concourse.bass_utils import run_bass_kernel_spmd
from concourse.tile import TileContext

BF16 = ml_dtypes.bfloat16

# ---------------- problem constants (full size, hardcoded) ----------------
N_NODES = 100000
N_EDGES = 1600000
IN_FEAT = 64
OUT_FEAT = 64
NUM_RELS = 8
NUM_BASES = 2
N_CORES = 8

WIN = 64              # dst nodes per window (= PSUM slot count)
BANK_BITS = 15        # int16 gather index reach


class Config:
    def __init__(self, n_nodes, n_cores=8, win=64, gw=12, sw=4,
                 bank_bits=BANK_BITS):
        assert gw % sw == 0, "S01 groups must nest inside gather groups"
        self.n_nodes = n_nodes
        self.bank_bits = bank_bits
        self.bank = 1 << bank_bits
        self.n_cores = n_cores
        self.win = win
        self.gw = gw                      # windows per gather group
        self.sw = sw                      # windows per S01-build group
        nw = -(-n_nodes // (n_cores * win))   # windows per core
        # round windows per core up to a multiple of gw
        nw = -(-nw // gw) * gw
        self.nw = nw
        self.npc = nw * win               # nodes per core (padded)
        self.nbank = -(-n_nodes // self.bank)
        self.ngroup = nw // gw


def make_schedule(cfg, src, dst, etypes, coeff):
    """Bucket edges by (core, window, bank); build per-core padded slot
    arrays in two orders:
      - gather order  : (group, bank, window)  -> int16 idx arrays
      - window order  : (window, bank)         -> dstl / c0 / c1 arrays
    Slot counts are shared across cores (max), so the SPMD program is
    uniform. Returns (sched_dict, per_core_input_arrays)."""
    K, NW, NB, W = cfg.n_cores, cfg.nw, cfg.nbank, cfg.win
    core = dst // cfg.npc
    w = (dst - core * cfg.npc) // W
    dstl = dst % W
    bank = src >> cfg.bank_bits
    idx16 = (src & (cfg.bank - 1)).astype(np.int16)
    cc = coeff[etypes].astype(np.float32)            # [E, 2]

    counts = np.zeros((K, NW, NB), np.int64)
    np.add.at(counts, (core, w, bank), 1)
    slots = 128 * np.ceil(counts.max(axis=0) / 128).astype(np.int64)  # [NW, NB]
    nt = slots // 128                                 # tiles per (w, b)
    tiles_w = nt.sum(axis=1)                          # tiles per window
    n_tiles = int(tiles_w.sum())
    n_slots = n_tiles * 128

    # window-major slot offset of bucket (w, b)
    woff = np.zeros((NW, NB), np.int64)
    flat = slots.reshape(-1)
    woff.reshape(-1)[1:] = np.cumsum(flat)[:-1]
    # gather-order slot offset of bucket (w, b): order (g, b, w in g)
    goff = np.zeros((NW, NB), np.int64)
    run = 0
    gchunk = np.zeros((NW, NB), np.int64)   # chunk offset within group tile
    for g in range(cfg.ngroup):
        grun = 0
        for b in range(NB):
            for wi in range(g * cfg.gw, (g + 1) * cfg.gw):
                goff[wi, b] = run
                gchunk[wi, b] = grun
                run += slots[wi, b]
                grun += slots[wi, b] // 128
    assert run == n_slots

    group_nidx = np.zeros((cfg.ngroup, NB), np.int64)
    for g in range(cfg.ngroup):
        for b in range(NB):
            group_nidx[g, b] = slots[g * cfg.gw:(g + 1) * cfg.gw, b].sum()
    group_chunks = group_nidx.sum(axis=1) // 128      # C_g per group

    # ---- per-core arrays ----
    per_core = []
    for k in range(K):
        m = core == k
        ew, eb = w[m], bank[m]
        edstl, eidx, ecc = dstl[m], idx16[m], cc[m]
        # order edges by (w, b) then place sequentially into both layouts
        order = np.lexsort((eb, ew))
        ew, eb, edstl, eidx, ecc = (
            ew[order], eb[order], edstl[order], eidx[order], ecc[order])
        # position of each edge within its bucket
        # edges are sorted by (w,b); per-bucket running index:
        bucket_id = ew * NB + eb
        # stable running counter per bucket
        pos = np.zeros(len(ew), np.int64)
        if len(ew):
            change = np.r_[True, bucket_id[1:] != bucket_id[:-1]]
            start_of_run = np.flatnonzero(change)
            run_id = np.cumsum(change) - 1
            pos = np.arange(len(ew)) - start_of_run[run_id]

        g_idx = np.zeros(n_slots, np.int16)           # gather order, pad idx 0
        wm_dstl = np.full(n_slots, W, np.float32)     # window order, pad -> W
        wm_cc = np.zeros((n_slots, 2), np.float32)

        gslot = goff[ew, eb] + pos
        wslot = woff[ew, eb] + pos
        g_idx[gslot] = eidx
        wm_dstl[wslot] = edstl
        wm_cc[wslot] = ecc

        # wrap gather indices: idx i -> [16, n/16] partition-wrapped, x8
        wrapped = g_idx.reshape(n_slots // 16, 16).T  # [16, n/16]
        wrapped = np.tile(wrapped, (8, 1))            # [128, n/16]

        # window-major per-slot metadata -> [128, n_tiles] layout
        # slot s = tile*128 + p  ->  partition p, column tile
        dstl_t = wm_dstl.reshape(n_tiles, 128).T.astype(BF16)      # [128, T]
        cc_t = np.ascontiguousarray(
            wm_cc.reshape(n_tiles, 128, 2).transpose(1, 0, 2)
        ).reshape(128, n_tiles * 2).astype(BF16)                   # [128, 2T]
        per_core.append({"idx": wrapped, "dstl": dstl_t, "cc": cc_t})

    sched = {
        "slots": slots, "nt": nt, "tiles_w": tiles_w, "n_tiles": n_tiles,
        "n_slots": n_slots, "woff": woff, "gchunk": gchunk,
        "group_nidx": group_nidx, "group_chunks": group_chunks,
    }
    return sched, per_core


def build_program(cfg, sched):
    import os
    dbg_stage = int(os.environ.get("K_STAGE", "9"))
    NW, NB, W = cfg.nw, cfg.nbank, cfg.win
    n_tiles, n_slots = sched["n_tiles"], sched["n_slots"]
    nt, tiles_w, woff = sched["nt"], sched["tiles_w"], sched["woff"]
    gchunk = sched["gchunk"]
    group_nidx, group_chunks = sched["group_nidx"], sched["group_chunks"]
    BANK = cfg.bank
    bankrows = [min(BANK, cfg.n_nodes - b * BANK) for b in range(NB)]

    nc = bacc.Bacc("TRN2", target_bir_lowering=False, debug=False,
                   num_devices=cfg.n_cores, num_swdge_queues=4)
    dt = mybir.dt

    table = nc.dram_tensor("table", [cfg.n_nodes, 128], dt.bfloat16,
                           kind="ExternalInput").ap()
    idx_d = nc.dram_tensor("idx", [128, n_slots // 16], dt.int16,
                           kind="ExternalInput").ap()
    dstl_d = nc.dram_tensor("dstl", [128, n_tiles], dt.bfloat16,
                            kind="ExternalInput").ap()
    cc_d = nc.dram_tensor("cc", [128, 2 * n_tiles], dt.bfloat16,
                          kind="ExternalInput").ap()
    featT_d = nc.dram_tensor("featT", [65, cfg.npc], dt.float32,
                             kind="ExternalInput").ap()
    wmat_d = nc.dram_tensor("wmat", [64, 2 * 64], dt.bfloat16,
                            kind="ExternalInput").ap()   # [d, (b,o)]
    lw_d = nc.dram_tensor("lw65", [65, 64], dt.float32,
                          kind="ExternalInput").ap()
    iota_d = nc.dram_tensor("iota", [128, W], dt.bfloat16,
                            kind="ExternalInput").ap()
    out_d = nc.dram_tensor("out", [cfg.npc, 64], dt.float32,
                           kind="ExternalOutput").ap()

    max_cg = int(group_chunks.max())
    max_gidx = int(group_nidx.sum(axis=1).max())
    sw_tiles = [int(tiles_w[s * cfg.sw:(s + 1) * cfg.sw].sum())
                for s in range(NW // cfg.sw)]
    max_st = max(sw_tiles) if sw_tiles else 0

    with TileContext(nc) as tc:
        with (
            tc.tile_pool(name="const", bufs=1) as cpool,
            tc.tile_pool(name="gather", bufs=2) as gpool,
            tc.tile_pool(name="gidx", bufs=2) as ipool,
            tc.tile_pool(name="sel", bufs=2) as spool,
            tc.tile_pool(name="acopy", bufs=3) as apool,
            tc.tile_pool(name="hout", bufs=3) as hpool,
            tc.tile_pool(name="psum_a", bufs=2, space="PSUM") as pa,
            tc.tile_pool(name="psum_h", bufs=2, space="PSUM") as ph,
        ):
            # resident constants / metadata
            dstl_t = cpool.tile([128, n_tiles], dt.bfloat16)
            nc.sync.dma_start(out=dstl_t[:], in_=dstl_d[:])
            cc_t = cpool.tile([128, 2 * n_tiles], dt.bfloat16)
            nc.sync.dma_start(out=cc_t[:], in_=cc_d[:])
            featT_t = cpool.tile([65, cfg.npc], dt.float32)
            nc.sync.dma_start(out=featT_t[:], in_=featT_d[:])
            wmat_t = cpool.tile([64, 2 * 64], dt.bfloat16)
            nc.sync.dma_start(out=wmat_t[:], in_=wmat_d[:])
            lw_t = cpool.tile([65, 64], dt.float32)
            nc.sync.dma_start(out=lw_t[:], in_=lw_d[:])
            iota_t = cpool.tile([128, W], dt.bfloat16)
            nc.sync.dma_start(out=iota_t[:], in_=iota_d[:])

            for g in range(cfg.ngroup if dbg_stage >= 1 else 0):
                cg = int(group_chunks[g])
                gt = gpool.tile([128, max_cg, 128], dt.bfloat16, tag="g")
                nidx_g = int(group_nidx[g].sum())
                it = ipool.tile([128, max_gidx // 16], dt.int16, tag="i")
                idx_off = int(group_nidx[:g].sum()) // 16
                nc.sync.dma_start(
                    out=it[:, : nidx_g // 16],
                    in_=idx_d[:, idx_off: idx_off + nidx_g // 16])
                # gather calls per bank, capped at MAX_GATHER idx per call
                MAX_GATHER = 8192
                coff = 0
                ioff = 0
                for b in range(NB):
                    nidx = int(group_nidx[g, b])
                    done = 0
                    while done < nidx:
                        n1 = min(MAX_GATHER, nidx - done)
                        nchunk = n1 // 128
                        nc.gpsimd.dma_gather(
                            out_ap=gt[:, coff: coff + nchunk, :],
                            in_ap=table[b * BANK: b * BANK + bankrows[b], :],
                            idxs_ap=it[:, ioff: ioff + n1 // 16],
                            num_idxs=n1,
                            num_idxs_reg=n1,
                            elem_size=128,
                            queue_num=b % 4,
                            single_packet=False,
                        )
                        coff += nchunk
                        ioff += n1 // 16
                        done += n1

                if dbg_stage < 2:
                    continue
                # process this group's windows
                for s in range(g * cfg.gw // cfg.sw,
                               (g + 1) * cfg.gw // cfg.sw):
                    w0 = s * cfg.sw
                    t0 = int(tiles_w[:w0].sum())
                    ts = int(tiles_w[w0: w0 + cfg.sw].sum())
                    if ts > 0:
                        onehot = spool.tile([128, max_st, W], dt.bfloat16,
                                            tag="oh")
                        s01 = spool.tile([128, max_st, 2, W], dt.bfloat16,
                                         tag="s01")
                        nc.vector.tensor_tensor(
                            out=onehot[:, :ts, :],
                            in0=dstl_t[:, t0: t0 + ts].unsqueeze(-1)
                                .to_broadcast([128, ts, W]),
                            in1=iota_t[:].unsqueeze(1)
                                .to_broadcast([128, ts, W]),
                            op=mybir.AluOpType.is_equal,
                        )
                        nc.vector.tensor_tensor(
                            out=s01[:, :ts, :, :],
                            in0=onehot[:, :ts, :].unsqueeze(2)
                                .to_broadcast([128, ts, 2, W]),
                            in1=cc_t[:, 2 * t0: 2 * (t0 + ts)]
                                .rearrange("p (t c) -> p t c", c=2)
                                .unsqueeze(-1).to_broadcast([128, ts, 2, W]),
                            op=mybir.AluOpType.mult,
                        )
                    if dbg_stage < 3:
                        continue
                    for wi in range(w0, w0 + cfg.sw):
                        tw = int(tiles_w[wi])
                        hps = ph.tile([64, 64], dt.float32, tag="h")
                        if tw > 0:
                            aps = pa.tile([64, 2 * W], dt.float32, tag="a")
                            ti = 0
                            for b in range(NB):
                                for j in range(int(nt[wi, b])):
                                    # window-major tile position rel. sgroup
                                    st_tile = int(woff[wi, b]) // 128 + j - t0
                                    rhs = s01[:, st_tile, :, :]\
                                        .rearrange("p c w -> p (c w)")
                                    nc.tensor.matmul(
                                        out=aps[:],
                                        lhsT=gt[:, int(gchunk[wi, b]) + j,
                                                0:64],
                                        rhs=rhs,
                                        start=(ti == 0),
                                        stop=(ti == tw - 1),
                                    )
                                    ti += 1
                            # agg [64d, (b,slot)] fp32 -> bf16 SBUF
                            aggs = apool.tile([64, 2 * W], dt.bfloat16,
                                              tag="agg")
                            nc.scalar.activation(
                                out=aggs[:], in_=aps[:],
                                func=mybir.ActivationFunctionType.Copy)
                        # h = sum_b agg_b^T(free) @ W_b  (+ selfloop w/ bias)
                        if tw > 0:
                            for b2 in range(2):
                                nc.tensor.matmul(
                                    out=hps[:],
                                    lhsT=aggs[:, b2 * W:(b2 + 1) * W],
                                    rhs=wmat_t[:, b2 * 64:(b2 + 1) * 64],
                                    start=(b2 == 0),
                                    stop=False,
                                )
                        nc.tensor.matmul(
                            out=hps[:],
                            lhsT=featT_t[:, wi * W: wi * W + 64],
                            rhs=lw_t[:],
                            start=(tw == 0),
                            stop=True,
                        )
                        hs = hpool.tile([64, 64], dt.float32, tag="hs")
                        nc.scalar.activation(
                            out=hs[:], in_=hps[:],
                            func=mybir.ActivationFunctionType.Copy)
                        nc.sync.dma_start(
                            out=out_d[wi * W: wi * W + 64, :], in_=hs[:])

    nc.compile()
    return nc


def make_inputs(cfg, per_core_sched, feat, W, coeff, h_bias, loop_weight):
    """Host-side tensor prep shared across cores + per-core metadata."""
    n = cfg.n_nodes
    table = np.zeros((n, 128), BF16)
    table[:, 0:64] = feat.astype(BF16)

    featT = np.zeros((65, cfg.npc), np.float32)
    featT[64, :] = 1.0
    ncore_nodes = min(cfg.npc, 10**18)
    # filled per core below
    wmat = np.ascontiguousarray(
        W.transpose(1, 0, 2).reshape(64, 2 * 64)).astype(BF16)  # [d,(b,o)]
    lw65 = np.concatenate(
        [loop_weight.astype(np.float32), h_bias[None].astype(np.float32)], 0)
    iota = np.tile(np.arange(cfg.win, dtype=np.float32)[None], (128, 1))\
        .astype(BF16)

    in_maps = []
    for k in range(cfg.n_cores):
        fT = featT.copy()
        lo = k * cfg.npc
        hi = min((k + 1) * cfg.npc, n)
        if hi > lo:
            fT[0:64, : hi - lo] = feat[lo:hi].T
        pc = per_core_sched[k]
        in_maps.append({
            "table": table,
            "idx": pc["idx"],
            "dstl": pc["dstl"],
            "cc": pc["cc"],
            "featT": fT,
            "wmat": wmat,
            "lw65": lw65,
            "iota": iota,
        })
    return in_maps


def run(cfg, feat, W, coeff, h_bias, loop_weight, src, dst, etypes,
        trace=False, sim=False):
    sched, per_core = make_schedule(
        cfg, src.astype(np.int64), dst.astype(np.int64),
        etypes.astype(np.int64), np.asarray(coeff, np.float32))
    nc = build_program(cfg, sched)
    in_maps = make_inputs(cfg, per_core, np.asarray(feat, np.float32),
                          np.asarray(W, np.float32),
                          np.asarray(coeff, np.float32),
                          np.asarray(h_bias, np.float32),
                          np.asarray(loop_weight, np.float32))
    if sim:
        import concourse.bass_interp as bass_interp
        msim = bass_interp.MultiCoreSim(nc, cfg.n_cores)
        for k in range(cfg.n_cores):
            for name, arr in in_maps[k].items():
                msim.cores[k].tensor(name)[:] = arr
        msim.simulate()
        outs = [np.array(msim.cores[k].tensor("out"))
                for k in range(cfg.n_cores)]
        h = np.concatenate(outs, axis=0)[: cfg.n_nodes]
        return h, None
    res = run_bass_kernel_spmd(nc, in_maps, list(range(cfg.n_cores)),
                               trace=trace)
    outs = [res.results[k]["out"] for k in range(cfg.n_cores)]
    h = np.concatenate(outs, axis=0)[: cfg.n_nodes]
    return h, res


def kernel(feat, W, coeff, h_bias, loop_weight, src, dst, etypes):
    cfg = Config(N_NODES)
    h, _ = run(cfg, feat, W, coeff, h_bias, loop_weight, src, dst, etypes)
    return h.astype(np.float32)
